# revision 1
# baseline (speedup 1.0000x reference)
"""JointRetention Trainium2 kernel.

out[b] = ((xpos(X_b Wq) xpos_down(X_b Wk)^T) * D[b%17]) @ (X_b Wv)

Strategy (v6):
  - Data-parallel over B*J=1088 across 8 cores (136 each; 136%17==0 so the
    joint index pattern is identical on every core).
  - Even/odd d-permutation: with head-dim columns reordered [even | odd],
    rotate_every_two becomes a half-swap, so xpos needs NO extra projection
    matmuls (removes the X@(W R) products = 8 matmuls/pair vs the rotation-
    folded-into-W formulation):
      Qx_e = Qe*hC - Qo*hS ;  Qx_o = Qo*hC + Qe*hS
  - f32r matmuls everywhere (1 cycle/col at >=256 free), f32 PSUM.
  - X host-packed (pair, p, b, lc, h) and padded so each pair is ONE 1MB
    input DMA and ONE output DMA (2 sync DMAs per pair vs 8).
  - Transposes on PE (f32, identity moving); projections read XT; DVE's
    xpos multiplies read the projection PSUM directly (no extra cast);
    gpsimd does the xpos combines; scalar does all psum->sbuf copies;
    scores/mask/out use full-bank [128,512] tiles (2 m-tiles per bank).
"""

import numpy as np

L = 243
H = 256
J = 17
NCORES = 8
NB = 1088
BPC = NB // NCORES          # 136 batch rows per core
NPAIR = BPC // 2            # 68 pairs per core
SCALE_BASE = 512
CHUNK = 81

_cache = {}


def _host_tables(W_Q, W_K, W_V, gamma):
    f32 = np.float32
    half = H // 2

    # even/odd permutation of head-dim columns
    pe = np.arange(0, H, 2)
    po = np.arange(1, H, 2)
    Wq = W_Q.astype(f32)
    Wk = W_K.astype(f32)
    Wv = W_V.astype(f32)
    # cols: [Qe | Qo | Ke | Ko | V]  (256 x 768)
    Wcat = np.concatenate(
        [Wq[:, pe], Wq[:, po], Wk[:, pe], Wk[:, po], Wv], axis=1)
    W_all = np.stack([Wcat[0:128], Wcat[128:256]], axis=0)  # (2,128,768)

    # half-tables (128, L) -> packed pair layout (128, 512)
    base_scale = ((np.arange(0, H, 2, dtype=f32) + 0.4 * H) / (1.4 * H)).astype(f32)
    pos = np.arange(L, dtype=f32)
    scale = base_scale[None, :] ** (pos / SCALE_BASE)[:, None]        # (L, half)
    inv_freq = (1.0 / 10000.0 ** (np.arange(half, dtype=f32) / half)).astype(f32)
    sinus = pos[:, None] * inv_freq[None, :]
    sin, cos = np.sin(sinus).astype(f32), np.cos(sinus).astype(f32)
    hCq = (cos * scale).T
    hSq = (sin * scale).T
    hCk = (cos / scale).T
    hSk = (sin / scale).T

    CS = np.zeros((4, 128, 512), f32)
    for i, tbl in enumerate([hCq, hSq, hCk, hSk]):
        CS[i, :, 0:L] = tbl
        CS[i, :, 256:256 + L] = tbl

    # decay mask, transposed per joint, packed [m-tile0 | m-tile1] in cols:
    # DTP[j][m', 0:256)    = D[j, l, m']        (m' in [0,128))
    # DTP[j][m', 256:512)  = D[j, l, 128+m']    (m' in [0,115))
    g = gamma.astype(f32)
    i = np.arange(L)[:, None]
    jj = np.arange(L)[None, :]
    allowed = jj < (i // CHUNK + 1) * CHUNK
    absd = np.abs(i - jj).astype(f32)
    D = g[:, None, None] ** absd[None]
    D = np.where(allowed[None], D, 0.0)
    D = np.where(np.isnan(D), 0.0, D).astype(f32)
    DT = np.transpose(D, (0, 2, 1))                       # (J, m, l)
    DTP = np.zeros((J, 128, 512), f32)
    DTP[:, :, 0:L] = DT[:, 0:128, :]
    DTP[:, 0:L - 128, 256:256 + L] = DT[:, 128:L, :]

    ident = np.eye(128, dtype=f32)
    return W_all, CS, DTP, ident


def _host_pack_x(Xc):
    # (BPC, 243, 256) f32 -> (NPAIR, 128, 1024) with cols (b, lc, h),
    # l-rows padded to 256
    Xp = np.zeros((BPC, 256, H), np.float32)
    Xp[:, :L] = Xc
    Xp = Xp.reshape(NPAIR, 2, 2, 128, H)          # t, b, lc, p, h
    Xp = np.transpose(Xp, (0, 3, 1, 2, 4))        # t, p, b, lc, h
    return np.ascontiguousarray(Xp.reshape(NPAIR, 128, 1024))


def _host_unpack_o(Oc):
    # (NPAIR, 128, 1024) -> (BPC, 243, 256)
    Op = Oc.reshape(NPAIR, 128, 2, 2, H)
    Op = np.transpose(Op, (0, 2, 3, 1, 4))        # t, b, lc, p, h
    Op = Op.reshape(BPC, 256, H)
    return np.ascontiguousarray(Op[:, :L])


def _build():
    import concourse.bacc as bacc
    import concourse.mybir as mybir
    from concourse import tile

    dt = mybir.dt
    f32 = dt.float32
    f32r = dt.float32r

    nc = bacc.Bacc("TRN2", target_bir_lowering=False, debug=False,
                   num_devices=NCORES)
    X_d = nc.dram_tensor("X", (NPAIR, 128, 1024), f32, kind="ExternalInput").ap()
    W_d = nc.dram_tensor("WALL", (2, 128, 768), f32, kind="ExternalInput").ap()
    CS_d = nc.dram_tensor("CS", (4, 128, 512), f32, kind="ExternalInput").ap()
    DT_d = nc.dram_tensor("DTAB", (J, 128, 512), f32, kind="ExternalInput").ap()
    ID_d = nc.dram_tensor("IDEN", (128, 128), f32, kind="ExternalInput").ap()
    O_d = nc.dram_tensor("OUT", (NPAIR, 128, 1024), f32, kind="ExternalOutput").ap()

    MSZ = (128, L - 128)          # m-tile sizes (128, 115)

    with tile.TileContext(nc) as tc:
        with (
            tc.tile_pool(name="const", bufs=1) as const,
            tc.tile_pool(name="xin", bufs=4) as xin,
            tc.tile_pool(name="work", bufs=4) as work,
            tc.tile_pool(name="pxt", bufs=2, space="PSUM") as pxt,
            tc.tile_pool(name="pproj", bufs=3, space="PSUM") as pproj,
            tc.tile_pool(name="psv", bufs=3, space="PSUM") as psv,
        ):
            # ---- constants ----
            w_f32 = [const.tile([128, 768], f32, name=f"wf{h}", tag=f"wf{h}")
                     for h in range(2)]
            w_r = [const.tile([128, 768], f32r, name=f"w{h}", tag=f"w{h}")
                   for h in range(2)]
            cs_sb = const.tile([128, 2048], f32, name="cs", tag="cs")
            dt_sb = [const.tile([128, 512], f32, name=f"dt{j}", tag=f"dt{j}")
                     for j in range(J)]
            ident = const.tile([128, 128], f32, name="ident", tag="ident")
            for h in range(2):
                nc.sync.dma_start(w_f32[h][:], W_d[h])
                nc.scalar.copy(w_r[h][:], w_f32[h][:])
            for i in range(4):
                nc.sync.dma_start(cs_sb[:, i * 512:(i + 1) * 512], CS_d[i])
            for j in range(J):
                nc.sync.dma_start(dt_sb[j][:], DT_d[j])
            nc.sync.dma_start(ident[:], ID_d[:])

            def load_x(t):
                xi = xin.tile([128, 1024], f32, name="xi", tag="xi")
                nc.sync.dma_start(xi[:], X_d[t])
                return xi

            xi_cur = load_x(0)

            for t in range(NPAIR):
                b0 = 2 * t
                joints = (b0 % J, (b0 + 1) % J)

                # ---- transpose X -> XT (h on partitions) via PE ----
                # xt cols: k*256 + l (l-pads 243..255 zero from host pad)
                xt_sb = []
                for hc in range(2):
                    ps = pxt.tile([128, 512], f32, name="xtp", tag="xtp")
                    for k in range(2):
                        for lc in range(2):
                            nc.tensor.transpose(
                                ps[:, k * 256 + lc * 128:
                                   k * 256 + lc * 128 + 128],
                                xi_cur[:, k * 512 + lc * 256 + hc * 128:
                                       k * 512 + lc * 256 + hc * 128 + 128],
                                ident[:],
                            )
                    sb = work.tile([128, 512], f32r, name=f"xt{hc}", tag=f"xt{hc}")
                    nc.scalar.copy(sb[:], ps[:])
                    xt_sb.append(sb)

                # prefetch next pair's X (xi_cur's last reader is above)
                if t + 1 < NPAIR:
                    xi_cur = load_x(t + 1)

                # ---- projections Qe,Qo,Ke,Ko; DVE xpos muls read the PSUM
                # directly; gpsimd combines ----
                #  Qx_e = Qe*hC - Qo*hS ; Qx_o = Qo*hC + Qe*hS (K likewise)
                qk = []
                for ti in range(2):          # 0=Q, 1=K
                    ctab = cs_sb[:, (2 * ti) * 512:(2 * ti) * 512 + 512]
                    stab = cs_sb[:, (2 * ti + 1) * 512:(2 * ti + 1) * 512 + 512]
                    tt = []
                    for half in range(2):    # 0=even, 1=odd
                        ps = pproj.tile([128, 512], f32, name="proj", tag="proj")
                        for hc in range(2):
                            nc.tensor.matmul(
                                ps[:],
                                w_r[hc][:, (2 * ti + half) * 128:
                                        (2 * ti + half) * 128 + 128],
                                xt_sb[hc][:],
                                start=(hc == 0), stop=(hc == 1),
                            )
                        ta = work.tile([128, 512], f32, name="ta", tag=f"ta{half}")
                        tb = work.tile([128, 512], f32, name="tb", tag=f"tb{half}")
                        # even proj: *hC -> xe-term, *hS -> xo-term
                        # odd  proj: *hS -> xe-term, *hC -> xo-term
                        nc.vector.tensor_mul(
                            ta[:], ps[:], ctab if half == 0 else stab)
                        nc.vector.tensor_mul(
                            tb[:], ps[:], stab if half == 0 else ctab)
                        tt.append((ta, tb))
                    xe = work.tile([128, 512], f32r, name=f"xe{ti}", tag=f"xe{ti}")
                    xo = work.tile([128, 512], f32r, name=f"xo{ti}", tag=f"xo{ti}")
                    nc.gpsimd.tensor_sub(xe[:], tt[0][0][:], tt[1][0][:])
                    nc.gpsimd.tensor_add(xo[:], tt[1][1][:], tt[0][1][:])
                    qk.append((xe, xo))
                (qx_e, qx_o), (kx_e, kx_o) = qk

                # ---- V = X @ Wv (natural layout; both m-tiles in one bank:
                # cols 0:256 = m in [0,128), cols 256:512 = m in [128,243)) ----
                v_sb = []
                for k in range(2):
                    ps = psv.tile([128, 512], f32, name="vps", tag="ps")
                    for mc in range(2):
                        msz = MSZ[mc]
                        for hc in range(2):
                            nc.tensor.matmul(
                                ps[0:msz, mc * 256:mc * 256 + 256],
                                xt_sb[hc][:, k * 256 + mc * 128:
                                          k * 256 + mc * 128 + msz],
                                w_r[hc][:, 512:768],
                                start=(hc == 0), stop=(hc == 1),
                            )
                    sb = work.tile([128, 512], f32r, name=f"v{k}", tag=f"v{k}")
                    nc.scalar.copy(sb[:], ps[:])
                    v_sb.append(sb)

                # ---- attention ----
                ob = work.tile([128, 1024], f32, name="ob", tag="ob")
                for k in range(2):
                    jt = joints[k]
                    # scores S^T, both m-tiles in one bank (256-col moving
                    # keeps f32r at 1 cycle/col; decay mask zeroes pads)
                    ps = psv.tile([128, 512], f32, name="sps", tag="ps")
                    for mc in range(2):
                        msz = MSZ[mc]
                        nc.tensor.matmul(
                            ps[0:msz, mc * 256:mc * 256 + 256],
                            kx_e[:, k * 256 + mc * 128:k * 256 + mc * 128 + msz],
                            qx_e[:, k * 256:k * 256 + 256],
                            start=True, stop=False)
                        nc.tensor.matmul(
                            ps[0:msz, mc * 256:mc * 256 + 256],
                            kx_o[:, k * 256 + mc * 128:k * 256 + mc * 128 + msz],
                            qx_o[:, k * 256:k * 256 + 256],
                            start=False, stop=True)
                    at = work.tile([128, 512], f32r, name=f"at{k}", tag=f"at{k}")
                    nc.vector.tensor_mul(at[:], ps[:], dt_sb[jt][:])

                    # out = A @ V
                    po = psv.tile([128, 512], f32, name="ops", tag="ps")
                    for lc in range(2):
                        lsz = MSZ[lc]
                        for mc in range(2):
                            nc.tensor.matmul(
                                po[0:lsz, lc * 256:lc * 256 + 256],
                                at[0:MSZ[mc], mc * 256 + lc * 128:
                                   mc * 256 + lc * 128 + lsz],
                                v_sb[k][0:MSZ[mc], mc * 256:mc * 256 + 256],
                                start=(mc == 0), stop=(mc == 1),
                            )
                    nc.scalar.copy(ob[:, k * 512:k * 512 + 512], po[:])

                nc.sync.dma_start(O_d[t], ob[:])

    nc.compile()
    return nc


def _get_nc():
    if "nc" not in _cache:
        _cache["nc"] = _build()
    return _cache["nc"]


def _run(in_maps, trace=False):
    from concourse import bass_utils
    nc = _get_nc()
    return bass_utils.run_bass_kernel_spmd(
        nc, in_maps, core_ids=list(range(NCORES)), trace=trace)


def kernel(X, W_Q, W_K, W_V, gamma, _trace=False):
    X = np.asarray(X, np.float32)
    W_all, CS, DTP, ident = _host_tables(
        np.asarray(W_Q, np.float32), np.asarray(W_K, np.float32),
        np.asarray(W_V, np.float32), np.asarray(gamma, np.float32))

    in_maps = []
    for c in range(NCORES):
        in_maps.append({
            "X": _host_pack_x(X[c * BPC:(c + 1) * BPC]),
            "WALL": W_all, "CS": CS, "DTAB": DTP, "IDEN": ident,
        })
    res = _run(in_maps, trace=_trace)
    out = np.concatenate([_host_unpack_o(r["OUT"]) for r in res.results],
                         axis=0)
    if _trace:
        _cache["last_result"] = res
    return out



# revision 3
# speedup vs baseline: 1.1449x; 1.1449x over previous
"""JointRetention Trainium2 kernel.

out[b] = ((xpos(X_b Wq) xpos_down(X_b Wk)^T) * D[b%17]) @ (X_b Wv)

Strategy (v7):
  - Data-parallel over B*J=1088 across 8 cores (136 each; 136%17==0 so the
    joint index pattern is identical on every core).
  - bf16 everywhere (tolerance 2e-2; measured ~1e-3): halves DMA, enables
    FWL weight loads, and 2x DVE perf mode on all SBUF elementwise ops.
  - X host-packed TRANSPOSED (h on partitions) so the PE does zero
    transposes; host also packs xpos cos/sin tables, fused decay tables.
  - Even/odd d-permutation: xpos becomes elementwise muls + half combines.
  - Chunk-sparse scores/AV: D[i,j]=0 for j >= (i//81+1)*81, so score
    m-tile1 skips i<81 and AV accumulates only live (l, m) chunks.
  - Engine balance per quad (4 batches): PE ~9us (proj/V/S/AV matmuls),
    ACT (Q/K/V psum->sbuf bf16 casts + half the out drains), DVE (xpos
    muls at 2x, decay mask, 1 combine, half the out drains), GPSIMD
    (3 of 4 combines).
"""

import numpy as np
from ml_dtypes import bfloat16

L = 243
LP = 244                     # l padded to even for DVE 2x inner dim
H = 256
J = 17
NCORES = 8
NB = 1088
BPC = NB // NCORES           # 136 batches per core
NPAIR = BPC // 2             # 68 pairs per core
NQUAD = NPAIR // 2           # 34 quads per core
SCALE_BASE = 512
CHUNK = 81

f32 = np.float32

_cache = {}


def _host_tables(W_Q, W_K, W_V, gamma):
    half = H // 2
    pe = np.arange(0, H, 2)
    po = np.arange(1, H, 2)
    Wcat = np.concatenate(
        [W_Q[:, pe], W_Q[:, po], W_K[:, pe], W_K[:, po], W_V], axis=1).astype(f32)
    W_all = np.stack([Wcat[0:128], Wcat[128:256]], axis=0)  # (2,128,768)

    base_scale = ((np.arange(0, H, 2, dtype=f32) + 0.4 * H) / (1.4 * H)).astype(f32)
    pos = np.arange(L, dtype=f32)
    scale = base_scale[None, :] ** (pos / SCALE_BASE)[:, None]
    inv_freq = (1.0 / 10000.0 ** (np.arange(half, dtype=f32) / half)).astype(f32)
    sinus = pos[:, None] * inv_freq[None, :]
    sin, cos = np.sin(sinus).astype(f32), np.cos(sinus).astype(f32)
    hCq = (cos * scale).T
    hSq = (sin * scale).T
    hCk = (cos / scale).T
    hSk = (sin / scale).T

    def padl(t):
        out = np.zeros((128, LP), f32)
        out[:, :L] = t
        return out

    def quad_tab(c, s):
        cp = np.concatenate([padl(c), padl(c)], axis=1)
        sp = np.concatenate([padl(s), padl(s)], axis=1)
        return np.concatenate([cp, sp, cp, sp], axis=1)   # (128, 1952)

    T_all = np.stack([quad_tab(hCq, hSq), quad_tab(hSq, hCq),
                      quad_tab(hCk, hSk), quad_tab(hSk, hCk)], axis=0)

    g = gamma.astype(f32)
    i = np.arange(L)[:, None]
    jj = np.arange(L)[None, :]
    allowed = jj < (i // CHUNK + 1) * CHUNK
    absd = np.abs(i - jj).astype(f32)
    D = g[:, None, None] ** absd[None]
    D = np.where(allowed[None], D, 0.0)
    D = np.where(np.isnan(D), 0.0, D).astype(f32)
    DTab = np.zeros((18, 128, 405), f32)
    for s in range(18):
        jt = s % J
        DTab[s, :, 0:L] = D[jt].T[0:128, :]
        DTab[s, 0:L - 128, L:405] = D[jt].T[128:L, 81:L]
    return (W_all.astype(bfloat16), T_all.astype(bfloat16),
            DTab.astype(bfloat16))


def _host_pack_x(Xc):
    # (BPC, 243, 256) f32 -> (NPAIR, 128, 1024) bf16, cols = hc*512+b*256+l
    Xp = Xc.reshape(NPAIR, 2, L, 2, 128)                  # pair, b, l, hc, p
    Xp = np.transpose(Xp, (0, 4, 3, 1, 2))                # pair, p, hc, b, l
    out = np.zeros((NPAIR, 128, 2, 2, 256), f32)
    out[:, :, :, :, 0:L] = Xp
    return np.ascontiguousarray(out.reshape(NPAIR, 128, 1024)).astype(bfloat16)


def _host_unpack_o(Oc):
    # (NPAIR, 128, 972) cols = b*486 + ht*243 + l -> (BPC, 243, 256) f32
    Op = Oc.astype(f32).reshape(NPAIR, 128, 2, 2, L)      # pair, p, b, ht, l
    Op = np.transpose(Op, (0, 2, 4, 3, 1))                # pair, b, l, ht, p
    return np.ascontiguousarray(Op.reshape(BPC, L, H))


def _build():
    import concourse.bacc as bacc
    import concourse.mybir as mybir
    from concourse import tile

    dt = mybir.dt
    F32 = dt.float32
    BF16 = dt.bfloat16
    MULT = mybir.AluOpType.mult
    ADD = mybir.AluOpType.add
    SUB = mybir.AluOpType.subtract

    nc = bacc.Bacc("TRN2", target_bir_lowering=False, debug=False,
                   num_devices=NCORES)
    X_d = nc.dram_tensor("X", (NPAIR, 128, 1024), BF16, kind="ExternalInput").ap()
    W_d = nc.dram_tensor("WALL", (2, 128, 768), BF16, kind="ExternalInput").ap()
    T_d = nc.dram_tensor("TTAB", (4, 128, 1952), BF16, kind="ExternalInput").ap()
    DT_d = nc.dram_tensor("DTAB", (18, 128, 405), BF16, kind="ExternalInput").ap()
    O_d = nc.dram_tensor("OUT", (NPAIR, 128, 972), BF16, kind="ExternalOutput").ap()

    with tile.TileContext(nc) as tc:
        with (
            tc.tile_pool(name="const", bufs=1) as const,
            tc.tile_pool(name="xin", bufs=4) as xin,
            tc.tile_pool(name="eo", bufs=2) as eo_pool,
            tc.tile_pool(name="tab", bufs=2) as tab_pool,
            tc.tile_pool(name="qx", bufs=2) as qx_pool,
            tc.tile_pool(name="vat", bufs=4) as vat,
            tc.tile_pool(name="osb", bufs=4) as osb_pool,
            tc.tile_pool(name="pqk", bufs=2, space="PSUM") as pqk,
            tc.tile_pool(name="pv", bufs=1, space="PSUM") as pv,
            tc.tile_pool(name="pattn", bufs=2, space="PSUM") as pattn,
        ):
            # ---- constants ----
            w_sb = [const.tile([128, 768], BF16, name=f"w{h}", tag=f"w{h}")
                    for h in range(2)]
            t_sb = [const.tile([128, 1952], BF16, name=f"t{i}", tag=f"t{i}")
                    for i in range(4)]
            dt_sb = const.tile([128, 18 * 405], BF16, name="dtab", tag="dtab")
            for h in range(2):
                nc.sync.dma_start(w_sb[h][:], W_d[h])
            for i in range(4):
                nc.sync.dma_start(t_sb[i][:], T_d[i])
            for s in range(18):
                nc.sync.dma_start(dt_sb[:, s * 405:(s + 1) * 405], DT_d[s])

            def load_x(t):
                xi = xin.tile([128, 1024], BF16, name="xi", tag="xi")
                nc.sync.dma_start(xi[:], X_d[t])
                return xi

            xt_cur = load_x(0)

            for q in range(NQUAD):
                qeo = eo_pool.tile([128, 1952], BF16, name="qeo", tag="qeo")
                keo = eo_pool.tile([128, 1952], BF16, name="keo", tag="keo")
                vsb = [None, None]
                for hh in range(2):
                    t = 2 * q + hh
                    xt = xt_cur

                    # ---- proj Q,K: psum [128,1024], e block 0:488, o 512:1000
                    pq_t = pqk.tile([128, 1024], F32, name="pq", tag="pqk")
                    pk_t = pqk.tile([128, 1024], F32, name="pk", tag="pqk")
                    for ti, ps in ((0, pq_t), (1, pk_t)):
                        for eo in range(2):
                            for hc in range(2):
                                mov = xt[:, hc * 512:hc * 512 + 512].rearrange(
                                    "p (b l) -> p b l", b=2)[:, :, 0:LP]
                                nc.tensor.matmul(
                                    ps[:, eo * 512:eo * 512 + 488],
                                    w_sb[hc][:, (ti * 2 + eo) * 128:
                                             (ti * 2 + eo) * 128 + 128],
                                    mov,
                                    start=(hc == 0), stop=(hc == 1),
                                )

                    # ---- V: psum [128,1024] = [b0 m0 | b0 m1 | b1 m0 | b1 m1]
                    pv_t = pv.tile([128, 1024], F32, name="pv", tag="pv")
                    for b in range(2):
                        for mc in range(2):
                            for hc in range(2):
                                off = hc * 512 + b * 256 + mc * 128
                                nc.tensor.matmul(
                                    pv_t[:, b * 512 + mc * 256:
                                         b * 512 + mc * 256 + 256],
                                    xt[:, off:off + 128],
                                    w_sb[hc][:, 512:768],
                                    start=(hc == 0), stop=(hc == 1),
                                )

                    # prefetch next pair's X
                    if t + 1 < NPAIR:
                        xt_cur = load_x(t + 1)

                    # ---- ACT drains (psum f32 -> sbuf bf16)
                    src = pq_t[:].rearrange("p (e c) -> p e c", e=2)[:, :, 0:488]
                    dst = qeo[:, hh * 976:hh * 976 + 976].rearrange(
                        "p (e c) -> p e c", e=2)
                    nc.scalar.copy(dst, src)
                    src = pk_t[:].rearrange("p (e c) -> p e c", e=2)[:, :, 0:488]
                    dst = keo[:, hh * 976:hh * 976 + 976].rearrange(
                        "p (e c) -> p e c", e=2)
                    nc.scalar.copy(dst, src)
                    vt = vat.tile([128, 1024], BF16, name="vsb", tag=f"vsb{hh}")
                    nc.scalar.copy(vt[:], pv_t[:])
                    vsb[hh] = vt

                # ---- xpos muls (DVE, bf16 2x) ----
                ta_q = tab_pool.tile([128, 1952], BF16, name="taq", tag="taq")
                tb_q = tab_pool.tile([128, 1952], BF16, name="tbq", tag="tbq")
                ta_k = tab_pool.tile([128, 1952], BF16, name="tak", tag="tak")
                tb_k = tab_pool.tile([128, 1952], BF16, name="tbk", tag="tbk")
                nc.vector.tensor_tensor(ta_q[:], qeo[:], t_sb[0][:], MULT)
                nc.vector.tensor_tensor(tb_q[:], qeo[:], t_sb[1][:], MULT)
                nc.vector.tensor_tensor(ta_k[:], keo[:], t_sb[2][:], MULT)
                nc.vector.tensor_tensor(tb_k[:], keo[:], t_sb[3][:], MULT)

                # ---- combines -> qx/kx [128,1024] 256-strided (pads stay 0)
                qx_e = qx_pool.tile([128, 1024], BF16, name="qxe", tag="qxe")
                qx_o = qx_pool.tile([128, 1024], BF16, name="qxo", tag="qxo")
                kx_e = qx_pool.tile([128, 1024], BF16, name="kxe", tag="kxe")
                kx_o = qx_pool.tile([128, 1024], BF16, name="kxo", tag="kxo")
                # l-pad columns feed S/AV stationary slices; keep them zero
                for z in (qx_e, qx_o, kx_e, kx_o):
                    nc.vector.memset(
                        z[:].rearrange("p (g l) -> p g l", g=4)[:, :, LP:256], 0.0)

                def c_src(tab, eo):
                    a = tab[:].rearrange("p (pr c) -> p pr c", pr=2)
                    a = a[:, :, eo * 488:(eo + 1) * 488]
                    return a.rearrange("p pr (b l) -> p pr b l", b=2)

                def c_dst(dst):
                    return dst[:].rearrange(
                        "p (pr b l) -> p pr b l", pr=2, b=2)[:, :, :, 0:LP]

                nc.vector.tensor_tensor(
                    c_dst(qx_e), c_src(ta_q, 0), c_src(ta_q, 1), SUB)
                nc.gpsimd.tensor_tensor(
                    c_dst(qx_o), c_src(tb_q, 1), c_src(tb_q, 0), ADD)
                nc.gpsimd.tensor_tensor(
                    c_dst(kx_e), c_src(ta_k, 0), c_src(ta_k, 1), SUB)
                nc.gpsimd.tensor_tensor(
                    c_dst(kx_o), c_src(tb_k, 1), c_src(tb_k, 0), ADD)

                # ---- attention per batch ----
                osb = [None, None]
                for b in range(4):
                    hh = b // 2
                    bl = b % 2
                    if bl == 0:
                        osb[hh] = osb_pool.tile([128, 972], BF16,
                                                name="ob", tag=f"ob{hh}")
                    boff = b * 256
                    # scores S^T [128, 405]: mt0 cols 0:243, mt1 243:405
                    ps = pattn.tile([128, 512], F32, name="sps", tag="attn")
                    nc.tensor.matmul(ps[:, 0:243],
                                     kx_e[:, boff:boff + 128],
                                     qx_e[:, boff:boff + 243],
                                     start=True, stop=False)
                    nc.tensor.matmul(ps[:, 0:243],
                                     kx_o[:, boff:boff + 128],
                                     qx_o[:, boff:boff + 243],
                                     start=False, stop=True)
                    nc.tensor.matmul(ps[:, 243:405],
                                     kx_e[:, boff + 128:boff + 256],
                                     qx_e[:, boff + 81:boff + 243],
                                     start=True, stop=False)
                    nc.tensor.matmul(ps[:, 243:405],
                                     kx_o[:, boff + 128:boff + 256],
                                     qx_o[:, boff + 81:boff + 243],
                                     start=False, stop=True)

                    # decay mask (DVE, psum 1x) -> A^T bf16
                    slot = (4 * q + b) % J
                    at = vat.tile([128, 416], BF16, name="at", tag=f"at{b % 2}")
                    nc.vector.tensor_tensor(
                        at[:, 0:405], ps[:, 0:405],
                        dt_sb[:, slot * 405:slot * 405 + 405], MULT)

                    # AV: out^T [128, 486] = [ht0 l 0:243 | ht1 l 0:243]
                    po = pattn.tile([128, 512], F32, name="ops", tag="attn")
                    v = vsb[hh]
                    for ht in range(2):
                        lhs0 = v[:, bl * 512 + ht * 128:bl * 512 + ht * 128 + 128]
                        lhs1 = v[:, bl * 512 + 256 + ht * 128:
                                 bl * 512 + 256 + ht * 128 + 128]
                        nc.tensor.matmul(po[:, ht * 243:ht * 243 + 81],
                                         lhs0, at[:, 0:81],
                                         start=True, stop=True)
                        nc.tensor.matmul(po[:, ht * 243 + 81:ht * 243 + 243],
                                         lhs0, at[:, 81:243],
                                         start=True, stop=False)
                        nc.tensor.matmul(po[:, ht * 243 + 81:ht * 243 + 243],
                                         lhs1, at[:, 243:405],
                                         start=False, stop=True)

                    # out drain: alternate ACT / DVE
                    dst = osb[hh][:, bl * 486:bl * 486 + 486]
                    if bl == 0:
                        nc.scalar.copy(dst, po[:, 0:486])
                    else:
                        nc.vector.tensor_copy(dst, po[:, 0:486])

                    if bl == 1:
                        nc.sync.dma_start(O_d[2 * q + hh], osb[hh][:])

    nc.compile()
    return nc


def _get_nc():
    if "nc" not in _cache:
        _cache["nc"] = _build()
    return _cache["nc"]


def _run(in_maps, trace=False):
    from concourse import bass_utils
    nc = _get_nc()
    return bass_utils.run_bass_kernel_spmd(
        nc, in_maps, core_ids=list(range(NCORES)), trace=trace)


def kernel(X, W_Q, W_K, W_V, gamma, _trace=False):
    X = np.asarray(X, f32)
    W_all, T_all, DTab = _host_tables(
        np.asarray(W_Q, f32), np.asarray(W_K, f32),
        np.asarray(W_V, f32), np.asarray(gamma, f32))

    in_maps = []
    for c in range(NCORES):
        in_maps.append({
            "X": _host_pack_x(X[c * BPC:(c + 1) * BPC]),
            "WALL": W_all, "TTAB": T_all, "DTAB": DTab,
        })
    res = _run(in_maps, trace=_trace)
    out = np.concatenate([_host_unpack_o(r["OUT"]) for r in res.results],
                         axis=0)
    if _trace:
        _cache["last_result"] = res
    return out.astype(f32)


# revision 5
# speedup vs baseline: 1.4437x; 1.2610x over previous
"""JointRetention Trainium2 kernel.

out[b] = ((xpos(X_b Wq) xpos_down(X_b Wk)^T) * D[b%17]) @ (X_b Wv)

Strategy (v7):
  - Data-parallel over B*J=1088 across 8 cores (136 each; 136%17==0 so the
    joint index pattern is identical on every core).
  - bf16 everywhere (tolerance 2e-2; measured ~1e-3): halves DMA, enables
    FWL weight loads, and 2x DVE perf mode on all SBUF elementwise ops.
  - X host-packed TRANSPOSED (h on partitions) so the PE does zero
    transposes; host also packs xpos cos/sin tables, fused decay tables.
  - Even/odd d-permutation: xpos becomes elementwise muls + half combines.
  - Chunk-sparse scores/AV: D[i,j]=0 for j >= (i//81+1)*81, so score
    m-tile1 skips i<81 and AV accumulates only live (l, m) chunks.
  - Engine balance per quad (4 batches): PE ~9us (proj/V/S/AV matmuls),
    ACT (Q/K/V psum->sbuf bf16 casts + half the out drains), DVE (xpos
    muls at 2x, decay mask, 1 combine, half the out drains), GPSIMD
    (3 of 4 combines).
"""

import numpy as np
from ml_dtypes import bfloat16

L = 243
LP = 244                     # l padded to even for DVE 2x inner dim
H = 256
J = 17
NCORES = 8
NB = 1088
BPC = NB // NCORES           # 136 batches per core
NPAIR = BPC // 2             # 68 pairs per core
NQUAD = NPAIR // 2           # 34 quads per core
SCALE_BASE = 512
CHUNK = 81

f32 = np.float32

_cache = {}


def _host_tables(W_Q, W_K, W_V, gamma):
    half = H // 2
    pe = np.arange(0, H, 2)
    po = np.arange(1, H, 2)
    Wcat = np.concatenate(
        [W_Q[:, pe], W_Q[:, po], W_K[:, pe], W_K[:, po], W_V], axis=1).astype(f32)
    W_all = np.stack([Wcat[0:128], Wcat[128:256]], axis=0)  # (2,128,768)

    base_scale = ((np.arange(0, H, 2, dtype=f32) + 0.4 * H) / (1.4 * H)).astype(f32)
    pos = np.arange(L, dtype=f32)
    scale = base_scale[None, :] ** (pos / SCALE_BASE)[:, None]
    inv_freq = (1.0 / 10000.0 ** (np.arange(half, dtype=f32) / half)).astype(f32)
    sinus = pos[:, None] * inv_freq[None, :]
    sin, cos = np.sin(sinus).astype(f32), np.cos(sinus).astype(f32)
    hCq = (cos * scale).T
    hSq = (sin * scale).T
    hCk = (cos / scale).T
    hSk = (sin / scale).T

    def padl(t):
        out = np.zeros((128, LP), f32)
        out[:, :L] = t
        return out

    def quad_tab(c, s):
        cp = np.concatenate([padl(c), padl(c)], axis=1)
        sp = np.concatenate([padl(s), padl(s)], axis=1)
        return np.concatenate([cp, sp, cp, sp], axis=1)   # (128, 1952)

    T_all = np.stack([quad_tab(hCq, hSq), quad_tab(hSq, hCq),
                      quad_tab(hCk, hSk), quad_tab(hSk, hCk)], axis=0)

    g = gamma.astype(f32)
    i = np.arange(L)[:, None]
    jj = np.arange(L)[None, :]
    allowed = jj < (i // CHUNK + 1) * CHUNK
    absd = np.abs(i - jj).astype(f32)
    D = g[:, None, None] ** absd[None]
    D = np.where(allowed[None], D, 0.0)
    D = np.where(np.isnan(D), 0.0, D).astype(f32)
    DTab = np.zeros((18, 128, 405), f32)
    for s in range(18):
        jt = s % J
        DTab[s, :, 0:L] = D[jt].T[0:128, :]
        DTab[s, 0:L - 128, L:405] = D[jt].T[128:L, 81:L]
    return (W_all.astype(bfloat16), T_all.astype(bfloat16),
            DTab.astype(bfloat16))


def _host_pack_x(Xc):
    # (BPC, 243, 256) f32 -> (NPAIR, 128, 1024) bf16, cols = hc*512+b*256+l
    Xp = Xc.reshape(NPAIR, 2, L, 2, 128)                  # pair, b, l, hc, p
    Xp = np.transpose(Xp, (0, 4, 3, 1, 2))                # pair, p, hc, b, l
    out = np.zeros((NPAIR, 128, 2, 2, 256), f32)
    out[:, :, :, :, 0:L] = Xp
    return np.ascontiguousarray(out.reshape(NPAIR, 128, 1024)).astype(bfloat16)


def _host_unpack_o(Oc):
    # (NPAIR, 128, 972) cols = b*486 + ht*243 + l -> (BPC, 243, 256) f32
    Op = Oc.astype(f32).reshape(NPAIR, 128, 2, 2, L)      # pair, p, b, ht, l
    Op = np.transpose(Op, (0, 2, 4, 3, 1))                # pair, b, l, ht, p
    return np.ascontiguousarray(Op.reshape(BPC, L, H))


def _build():
    import concourse.bacc as bacc
    import concourse.mybir as mybir
    from concourse import tile

    dt = mybir.dt
    F32 = dt.float32
    BF16 = dt.bfloat16
    MULT = mybir.AluOpType.mult
    ADD = mybir.AluOpType.add
    SUB = mybir.AluOpType.subtract

    nc = bacc.Bacc("TRN2", target_bir_lowering=False, debug=False,
                   num_devices=NCORES)
    X_d = nc.dram_tensor("X", (NPAIR, 128, 1024), BF16, kind="ExternalInput").ap()
    W_d = nc.dram_tensor("WALL", (2, 128, 768), BF16, kind="ExternalInput").ap()
    T_d = nc.dram_tensor("TTAB", (4, 128, 1952), BF16, kind="ExternalInput").ap()
    DT_d = nc.dram_tensor("DTAB", (18, 128, 405), BF16, kind="ExternalInput").ap()
    O_d = nc.dram_tensor("OUT", (NPAIR, 128, 972), BF16, kind="ExternalOutput").ap()

    with tile.TileContext(nc) as tc:
        with (
            tc.tile_pool(name="const", bufs=1) as const,
            tc.tile_pool(name="xin", bufs=4) as xin,
            tc.tile_pool(name="eo", bufs=2) as eo_pool,
            tc.tile_pool(name="tab", bufs=2) as tab_pool,
            tc.tile_pool(name="qx", bufs=2) as qx_pool,
            tc.tile_pool(name="vat", bufs=4) as vat,
            tc.tile_pool(name="osb", bufs=4) as osb_pool,
            tc.tile_pool(name="pqk", bufs=2, space="PSUM") as pqk,
            tc.tile_pool(name="pv", bufs=1, space="PSUM") as pv,
            tc.tile_pool(name="pattn", bufs=2, space="PSUM") as pattn,
        ):
            # ---- constants ----
            w_sb = [const.tile([128, 768], BF16, name=f"w{h}", tag=f"w{h}")
                    for h in range(2)]
            t_sb = [const.tile([128, 1952], BF16, name=f"t{i}", tag=f"t{i}")
                    for i in range(4)]
            dt_sb = const.tile([128, 18 * 405], BF16, name="dtab", tag="dtab")
            for h in range(2):
                nc.sync.dma_start(w_sb[h][:], W_d[h])
            for i in range(4):
                nc.sync.dma_start(t_sb[i][:], T_d[i])
            for s in range(18):
                nc.sync.dma_start(dt_sb[:, s * 405:(s + 1) * 405], DT_d[s])

            def load_x(t):
                xi = xin.tile([128, 1024], BF16, name="xi", tag="xi")
                nc.sync.dma_start(xi[:], X_d[t])
                return xi

            xt_cur = load_x(0)

            for q in range(NQUAD):
                qeo = eo_pool.tile([128, 1952], BF16, name="qeo", tag="qeo")
                keo = eo_pool.tile([128, 1952], BF16, name="keo", tag="keo")
                vsb = [None, None]
                for hh in range(2):
                    t = 2 * q + hh
                    xt = xt_cur

                    # ---- proj Q,K: psum [128,1024], e block 0:488, o 512:1000
                    pq_t = pqk.tile([128, 1024], F32, name="pq", tag="pqk")
                    pk_t = pqk.tile([128, 1024], F32, name="pk", tag="pqk")
                    for ti, ps in ((0, pq_t), (1, pk_t)):
                        for eo in range(2):
                            for hc in range(2):
                                mov = xt[:, hc * 512:hc * 512 + 512].rearrange(
                                    "p (b l) -> p b l", b=2)[:, :, 0:LP]
                                nc.tensor.matmul(
                                    ps[:, eo * 512:eo * 512 + 488],
                                    w_sb[hc][:, (ti * 2 + eo) * 128:
                                             (ti * 2 + eo) * 128 + 128],
                                    mov,
                                    start=(hc == 0), stop=(hc == 1),
                                )

                    # ---- V: psum [128,1024] = [b0 m0 | b0 m1 | b1 m0 | b1 m1]
                    pv_t = pv.tile([128, 1024], F32, name="pv", tag="pv")
                    for b in range(2):
                        for mc in range(2):
                            for hc in range(2):
                                off = hc * 512 + b * 256 + mc * 128
                                nc.tensor.matmul(
                                    pv_t[:, b * 512 + mc * 256:
                                         b * 512 + mc * 256 + 256],
                                    xt[:, off:off + 128],
                                    w_sb[hc][:, 512:768],
                                    start=(hc == 0), stop=(hc == 1),
                                )

                    # prefetch next pair's X
                    if t + 1 < NPAIR:
                        xt_cur = load_x(t + 1)

                    # ---- ACT drains (psum f32 -> sbuf bf16)
                    src = pq_t[:].rearrange("p (e c) -> p e c", e=2)[:, :, 0:488]
                    dst = qeo[:, hh * 976:hh * 976 + 976].rearrange(
                        "p (e c) -> p e c", e=2)
                    nc.scalar.copy(dst, src)
                    src = pk_t[:].rearrange("p (e c) -> p e c", e=2)[:, :, 0:488]
                    dst = keo[:, hh * 976:hh * 976 + 976].rearrange(
                        "p (e c) -> p e c", e=2)
                    nc.scalar.copy(dst, src)
                    vt = vat.tile([128, 1024], BF16, name="vsb", tag=f"vsb{hh}")
                    nc.scalar.copy(vt[:], pv_t[:])
                    vsb[hh] = vt

                # ---- xpos muls (DVE, bf16 2x) ----
                ta_q = tab_pool.tile([128, 1952], BF16, name="taq", tag="taq")
                tb_q = tab_pool.tile([128, 1952], BF16, name="tbq", tag="tbq")
                ta_k = tab_pool.tile([128, 1952], BF16, name="tak", tag="tak")
                tb_k = tab_pool.tile([128, 1952], BF16, name="tbk", tag="tbk")
                nc.vector.tensor_tensor(ta_q[:], qeo[:], t_sb[0][:], MULT)
                nc.vector.tensor_tensor(tb_q[:], qeo[:], t_sb[1][:], MULT)
                nc.vector.tensor_tensor(ta_k[:], keo[:], t_sb[2][:], MULT)
                nc.vector.tensor_tensor(tb_k[:], keo[:], t_sb[3][:], MULT)

                # ---- combines -> qx/kx [128,1024] 256-strided (pads stay 0)
                qx_e = qx_pool.tile([128, 1024], BF16, name="qxe", tag="qxe")
                qx_o = qx_pool.tile([128, 1024], BF16, name="qxo", tag="qxo")
                kx_e = qx_pool.tile([128, 1024], BF16, name="kxe", tag="kxe")
                kx_o = qx_pool.tile([128, 1024], BF16, name="kxo", tag="kxo")
                # l-pad columns feed S/AV stationary slices; keep them zero
                for z in (qx_e, qx_o, kx_e, kx_o):
                    nc.vector.memset(
                        z[:].rearrange("p (g l) -> p g l", g=4)[:, :, LP:256], 0.0)

                def c_src(tab, eo):
                    a = tab[:].rearrange("p (pr c) -> p pr c", pr=2)
                    a = a[:, :, eo * 488:(eo + 1) * 488]
                    return a.rearrange("p pr (b l) -> p pr b l", b=2)

                def c_dst(dst):
                    return dst[:].rearrange(
                        "p (pr b l) -> p pr b l", pr=2, b=2)[:, :, :, 0:LP]

                # all combines on DVE: gpsimd elementwise halves DVE
                # throughput via the shared SBUF port (measured)
                nc.vector.tensor_tensor(
                    c_dst(qx_e), c_src(ta_q, 0), c_src(ta_q, 1), SUB)
                nc.vector.tensor_tensor(
                    c_dst(qx_o), c_src(tb_q, 1), c_src(tb_q, 0), ADD)
                nc.vector.tensor_tensor(
                    c_dst(kx_e), c_src(ta_k, 0), c_src(ta_k, 1), SUB)
                nc.vector.tensor_tensor(
                    c_dst(kx_o), c_src(tb_k, 1), c_src(tb_k, 0), ADD)

                # ---- attention per batch ----
                osb = [None, None]
                for b in range(4):
                    hh = b // 2
                    bl = b % 2
                    if bl == 0:
                        osb[hh] = osb_pool.tile([128, 972], BF16,
                                                name="ob", tag=f"ob{hh}")
                    boff = b * 256
                    # scores S^T [128, 405]: mt0 cols 0:243, mt1 243:405
                    ps = pattn.tile([128, 512], F32, name="sps", tag="attn")
                    nc.tensor.matmul(ps[:, 0:243],
                                     kx_e[:, boff:boff + 128],
                                     qx_e[:, boff:boff + 243],
                                     start=True, stop=False)
                    nc.tensor.matmul(ps[:, 0:243],
                                     kx_o[:, boff:boff + 128],
                                     qx_o[:, boff:boff + 243],
                                     start=False, stop=True)
                    nc.tensor.matmul(ps[:, 243:405],
                                     kx_e[:, boff + 128:boff + 256],
                                     qx_e[:, boff + 81:boff + 243],
                                     start=True, stop=False)
                    nc.tensor.matmul(ps[:, 243:405],
                                     kx_o[:, boff + 128:boff + 256],
                                     qx_o[:, boff + 81:boff + 243],
                                     start=False, stop=True)

                    # decay mask (DVE, psum 1x) -> A^T bf16
                    slot = (4 * q + b) % J
                    at = vat.tile([128, 416], BF16, name="at", tag=f"at{b % 2}")
                    nc.vector.tensor_tensor(
                        at[:, 0:405], ps[:, 0:405],
                        dt_sb[:, slot * 405:slot * 405 + 405], MULT)

                    # AV: out^T [128, 486] = [ht0 l 0:243 | ht1 l 0:243]
                    po = pattn.tile([128, 512], F32, name="ops", tag="attn")
                    v = vsb[hh]
                    for ht in range(2):
                        lhs0 = v[:, bl * 512 + ht * 128:bl * 512 + ht * 128 + 128]
                        lhs1 = v[:, bl * 512 + 256 + ht * 128:
                                 bl * 512 + 256 + ht * 128 + 128]
                        nc.tensor.matmul(po[:, ht * 243:ht * 243 + 243],
                                         lhs0, at[:, 0:243],
                                         start=True, stop=False)
                        nc.tensor.matmul(po[:, ht * 243 + 81:ht * 243 + 243],
                                         lhs1, at[:, 243:405],
                                         start=False, stop=True)

                    # out drain: 3 of 4 on ACT, 1 on DVE
                    dst = osb[hh][:, bl * 486:bl * 486 + 486]
                    if b == 3:
                        nc.vector.tensor_copy(dst, po[:, 0:486])
                    else:
                        nc.scalar.copy(dst, po[:, 0:486])

                    if bl == 1:
                        nc.sync.dma_start(O_d[2 * q + hh], osb[hh][:])

    nc.compile()
    return nc


def _get_nc():
    if "nc" not in _cache:
        _cache["nc"] = _build()
    return _cache["nc"]


def _run(in_maps, trace=False):
    from concourse import bass_utils
    nc = _get_nc()
    return bass_utils.run_bass_kernel_spmd(
        nc, in_maps, core_ids=list(range(NCORES)), trace=trace)


def kernel(X, W_Q, W_K, W_V, gamma, _trace=False):
    X = np.asarray(X, f32)
    W_all, T_all, DTab = _host_tables(
        np.asarray(W_Q, f32), np.asarray(W_K, f32),
        np.asarray(W_V, f32), np.asarray(gamma, f32))

    in_maps = []
    for c in range(NCORES):
        in_maps.append({
            "X": _host_pack_x(X[c * BPC:(c + 1) * BPC]),
            "WALL": W_all, "TTAB": T_all, "DTAB": DTab,
        })
    res = _run(in_maps, trace=_trace)
    out = np.concatenate([_host_unpack_o(r["OUT"]) for r in res.results],
                         axis=0)
    if _trace:
        _cache["last_result"] = res
    return out.astype(f32)


# revision 9
# speedup vs baseline: 1.5233x; 1.0551x over previous
"""JointRetention Trainium2 kernel.

out[b] = ((xpos(X_b Wq) xpos_down(X_b Wk)^T) * D[b%17]) @ (X_b Wv)

Strategy (v7):
  - Data-parallel over B*J=1088 across 8 cores (136 each; 136%17==0 so the
    joint index pattern is identical on every core).
  - bf16 everywhere (tolerance 2e-2; measured ~1e-3): halves DMA, enables
    FWL weight loads, and 2x DVE perf mode on all SBUF elementwise ops.
  - X host-packed TRANSPOSED (h on partitions) so the PE does zero
    transposes; host also packs xpos cos/sin tables, fused decay tables.
  - Even/odd d-permutation: xpos becomes elementwise muls + half combines.
  - Chunk-sparse scores/AV: D[i,j]=0 for j >= (i//81+1)*81, so score
    m-tile1 skips i<81 and AV accumulates only live (l, m) chunks.
  - Engine balance per quad (4 batches): PE ~9us (proj/V/S/AV matmuls),
    ACT (Q/K/V psum->sbuf bf16 casts + half the out drains), DVE (xpos
    muls at 2x, decay mask, 1 combine, half the out drains), GPSIMD
    (3 of 4 combines).
"""

import numpy as np
from ml_dtypes import bfloat16

L = 243
LP = 244                     # l padded to even for DVE 2x inner dim
H = 256
J = 17
NCORES = 8
NB = 1088
BPC = NB // NCORES           # 136 batches per core
NPAIR = BPC // 2             # 68 pairs per core
NQUAD = NPAIR // 2           # 34 quads per core
SCALE_BASE = 512
CHUNK = 81

f32 = np.float32

_cache = {}


def _host_tables(W_Q, W_K, W_V, gamma):
    half = H // 2
    pe = np.arange(0, H, 2)
    po = np.arange(1, H, 2)
    Wcat = np.concatenate(
        [W_Q[:, pe], W_Q[:, po], W_K[:, pe], W_K[:, po], W_V], axis=1).astype(f32)
    W_all = np.stack([Wcat[0:128], Wcat[128:256]], axis=0)  # (2,128,768)

    base_scale = ((np.arange(0, H, 2, dtype=f32) + 0.4 * H) / (1.4 * H)).astype(f32)
    pos = np.arange(L, dtype=f32)
    scale = base_scale[None, :] ** (pos / SCALE_BASE)[:, None]
    inv_freq = (1.0 / 10000.0 ** (np.arange(half, dtype=f32) / half)).astype(f32)
    sinus = pos[:, None] * inv_freq[None, :]
    sin, cos = np.sin(sinus).astype(f32), np.cos(sinus).astype(f32)
    hCq = (cos * scale).T
    hSq = (sin * scale).T
    hCk = (cos / scale).T
    hSk = (sin / scale).T

    def padl(t):
        out = np.zeros((128, LP), f32)
        out[:, :L] = t
        return out

    def quad_tab(c, s):
        cp = np.concatenate([padl(c), padl(c)], axis=1)
        sp = np.concatenate([padl(s), padl(s)], axis=1)
        return np.concatenate([cp, sp, cp, sp], axis=1)   # (128, 1952)

    T_all = np.stack([quad_tab(hCq, hSq), quad_tab(hSq, hCq),
                      quad_tab(hCk, hSk), quad_tab(hSk, hCk)], axis=0)

    g = gamma.astype(f32)
    i = np.arange(L)[:, None]
    jj = np.arange(L)[None, :]
    allowed = jj < (i // CHUNK + 1) * CHUNK
    absd = np.abs(i - jj).astype(f32)
    D = g[:, None, None] ** absd[None]
    D = np.where(allowed[None], D, 0.0)
    D = np.where(np.isnan(D), 0.0, D).astype(f32)
    DTab = np.zeros((18, 128, 405), f32)
    for s in range(18):
        jt = s % J
        DTab[s, :, 0:L] = D[jt].T[0:128, :]
        DTab[s, 0:L - 128, L:405] = D[jt].T[128:L, 81:L]
    return (W_all.astype(bfloat16), T_all.astype(bfloat16),
            DTab.astype(bfloat16))


def _host_pack_x(Xc):
    # (BPC, 243, 256) f32 -> (NPAIR, 128, 1024) bf16, cols = hc*512+b*256+l
    Xp = Xc.reshape(NPAIR, 2, L, 2, 128)                  # pair, b, l, hc, p
    Xp = np.transpose(Xp, (0, 4, 3, 1, 2))                # pair, p, hc, b, l
    out = np.zeros((NPAIR, 128, 2, 2, 256), f32)
    out[:, :, :, :, 0:L] = Xp
    return np.ascontiguousarray(out.reshape(NPAIR, 128, 1024)).astype(bfloat16)


def _host_unpack_o(Oc):
    # (NPAIR, 128, 972) cols = b*486 + ht*243 + l -> (BPC, 243, 256) f32
    Op = Oc.astype(f32).reshape(NPAIR, 128, 2, 2, L)      # pair, p, b, ht, l
    Op = np.transpose(Op, (0, 2, 4, 3, 1))                # pair, b, l, ht, p
    return np.ascontiguousarray(Op.reshape(BPC, L, H))


def _build():
    import concourse.bacc as bacc
    import concourse.mybir as mybir
    from concourse import tile

    dt = mybir.dt
    F32 = dt.float32
    BF16 = dt.bfloat16
    MULT = mybir.AluOpType.mult
    ADD = mybir.AluOpType.add
    SUB = mybir.AluOpType.subtract

    nc = bacc.Bacc("TRN2", target_bir_lowering=False, debug=False,
                   num_devices=NCORES)
    X_d = nc.dram_tensor("X", (NPAIR, 128, 1024), BF16, kind="ExternalInput").ap()
    W_d = nc.dram_tensor("WALL", (2, 128, 768), BF16, kind="ExternalInput").ap()
    T_d = nc.dram_tensor("TTAB", (4, 128, 1952), BF16, kind="ExternalInput").ap()
    DT_d = nc.dram_tensor("DTAB", (18, 128, 405), BF16, kind="ExternalInput").ap()
    O_d = nc.dram_tensor("OUT", (NPAIR, 128, 972), BF16, kind="ExternalOutput").ap()

    with tile.TileContext(nc) as tc:
        with (
            tc.tile_pool(name="const", bufs=1) as const,
            tc.tile_pool(name="xin", bufs=6) as xin,
            tc.tile_pool(name="eo", bufs=3) as eo_pool,
            tc.tile_pool(name="tab", bufs=2) as tab_pool,
            tc.tile_pool(name="qx", bufs=3) as qx_pool,
            tc.tile_pool(name="vat", bufs=4) as vat,
            tc.tile_pool(name="osb", bufs=4) as osb_pool,
            tc.tile_pool(name="pqk", bufs=2, space="PSUM") as pqk,
            tc.tile_pool(name="pattn", bufs=4, space="PSUM") as pattn,
        ):
            # ---- constants ----
            w_sb = [const.tile([128, 768], BF16, name=f"w{h}", tag=f"w{h}")
                    for h in range(2)]
            t_sb = [const.tile([128, 1952], BF16, name=f"t{i}", tag=f"t{i}")
                    for i in range(4)]
            dt_sb = const.tile([128, 18 * 405], BF16, name="dtab", tag="dtab")
            for h in range(2):
                nc.sync.dma_start(w_sb[h][:], W_d[h])
            for i in range(4):
                nc.sync.dma_start(t_sb[i][:], T_d[i])
            for s in range(18):
                nc.sync.dma_start(dt_sb[:, s * 405:(s + 1) * 405], DT_d[s])

            def load_x(t):
                xi = xin.tile([128, 1024], BF16, name="xi", tag="xi")
                nc.sync.dma_start(xi[:], X_d[t])
                return xi

            xt_cur = load_x(0)

            for q in range(NQUAD):
                qeo = eo_pool.tile([128, 1952], BF16, name="qeo", tag="qeo")
                keo = eo_pool.tile([128, 1952], BF16, name="keo", tag="keo")
                vsb = [None, None]
                for hh in range(2):
                    t = 2 * q + hh
                    xt = xt_cur

                    # ---- proj Q,K: psum [128,1024], e block 0:488, o 512:1000
                    pq_t = pqk.tile([128, 1024], F32, name="pq", tag="pqk")
                    pk_t = pqk.tile([128, 1024], F32, name="pk", tag="pqk")
                    for ti, ps in ((0, pq_t), (1, pk_t)):
                        for eo in range(2):
                            for hc in range(2):
                                mov = xt[:, hc * 512:hc * 512 + 512].rearrange(
                                    "p (b l) -> p b l", b=2)[:, :, 0:LP]
                                nc.tensor.matmul(
                                    ps[:, eo * 512:eo * 512 + 488],
                                    w_sb[hc][:, (ti * 2 + eo) * 128:
                                             (ti * 2 + eo) * 128 + 128],
                                    mov,
                                    start=(hc == 0), stop=(hc == 1),
                                )

                    # ---- V: psum [128,1024] = [b0 m0 | b0 m1 | b1 m0 | b1 m1]
                    pv_t = pqk.tile([128, 1024], F32, name="pv", tag="pqk")
                    for b in range(2):
                        for mc in range(2):
                            for hc in range(2):
                                off = hc * 512 + b * 256 + mc * 128
                                nc.tensor.matmul(
                                    pv_t[:, b * 512 + mc * 256:
                                         b * 512 + mc * 256 + 256],
                                    xt[:, off:off + 128],
                                    w_sb[hc][:, 512:768],
                                    start=(hc == 0), stop=(hc == 1),
                                )

                    # prefetch next pair's X
                    if t + 1 < NPAIR:
                        xt_cur = load_x(t + 1)

                    # ---- ACT drains (psum f32 -> sbuf bf16)
                    src = pq_t[:].rearrange("p (e c) -> p e c", e=2)[:, :, 0:488]
                    dst = qeo[:, hh * 976:hh * 976 + 976].rearrange(
                        "p (e c) -> p e c", e=2)
                    nc.scalar.copy(dst, src)
                    src = pk_t[:].rearrange("p (e c) -> p e c", e=2)[:, :, 0:488]
                    dst = keo[:, hh * 976:hh * 976 + 976].rearrange(
                        "p (e c) -> p e c", e=2)
                    nc.scalar.copy(dst, src)
                    vt = vat.tile([128, 1024], BF16, name="vsb", tag=f"vsb{hh}")
                    nc.scalar.copy(vt[:], pv_t[:])
                    vsb[hh] = vt

                # ---- xpos muls (DVE, bf16 2x) ----
                ta_q = tab_pool.tile([128, 1952], BF16, name="taq", tag="taq")
                tb_q = tab_pool.tile([128, 1952], BF16, name="tbq", tag="tbq")
                ta_k = tab_pool.tile([128, 1952], BF16, name="tak", tag="tak")
                tb_k = tab_pool.tile([128, 1952], BF16, name="tbk", tag="tbk")
                nc.vector.tensor_tensor(ta_q[:], qeo[:], t_sb[0][:], MULT)
                nc.vector.tensor_tensor(tb_q[:], qeo[:], t_sb[1][:], MULT)
                nc.vector.tensor_tensor(ta_k[:], keo[:], t_sb[2][:], MULT)
                nc.vector.tensor_tensor(tb_k[:], keo[:], t_sb[3][:], MULT)

                # ---- combines -> qx/kx [128,1024] 256-strided (pads stay 0)
                qx_e = qx_pool.tile([128, 1024], BF16, name="qxe", tag="qxe")
                qx_o = qx_pool.tile([128, 1024], BF16, name="qxo", tag="qxo")
                kx_e = qx_pool.tile([128, 1024], BF16, name="kxe", tag="kxe")
                kx_o = qx_pool.tile([128, 1024], BF16, name="kxo", tag="kxo")
                # l-pad columns of kx feed S stationary slices; keep them zero
                # (qx is only ever read as moving operand over real columns)
                for z in (kx_e, kx_o):
                    nc.vector.memset(
                        z[:].rearrange("p (g l) -> p g l", g=4)[:, :, LP:256], 0.0)

                def c_src(tab, eo):
                    a = tab[:].rearrange("p (pr c) -> p pr c", pr=2)
                    a = a[:, :, eo * 488:(eo + 1) * 488]
                    return a.rearrange("p pr (b l) -> p pr b l", b=2)

                def c_dst(dst):
                    return dst[:].rearrange(
                        "p (pr b l) -> p pr b l", pr=2, b=2)[:, :, :, 0:LP]

                # all combines on DVE: gpsimd elementwise halves DVE
                # throughput via the shared SBUF port (measured)
                nc.vector.tensor_tensor(
                    c_dst(qx_e), c_src(ta_q, 0), c_src(ta_q, 1), SUB)
                nc.vector.tensor_tensor(
                    c_dst(qx_o), c_src(tb_q, 1), c_src(tb_q, 0), ADD)
                nc.vector.tensor_tensor(
                    c_dst(kx_e), c_src(ta_k, 0), c_src(ta_k, 1), SUB)
                nc.vector.tensor_tensor(
                    c_dst(kx_o), c_src(tb_k, 1), c_src(tb_k, 0), ADD)

                # ---- attention per batch ----
                osb = [None, None]
                for b in range(4):
                    hh = b // 2
                    bl = b % 2
                    if bl == 0:
                        osb[hh] = osb_pool.tile([128, 972], BF16,
                                                name="ob", tag=f"ob{hh}")
                    boff = b * 256
                    # scores S^T [128, 405]: mt0 cols 0:243, mt1 243:405
                    ps = pattn.tile([128, 512], F32, name="sps", tag="attn")
                    nc.tensor.matmul(ps[:, 0:243],
                                     kx_e[:, boff:boff + 128],
                                     qx_e[:, boff:boff + 243],
                                     start=True, stop=False)
                    nc.tensor.matmul(ps[:, 0:243],
                                     kx_o[:, boff:boff + 128],
                                     qx_o[:, boff:boff + 243],
                                     start=False, stop=True)
                    nc.tensor.matmul(ps[:, 243:405],
                                     kx_e[:, boff + 128:boff + 256],
                                     qx_e[:, boff + 81:boff + 243],
                                     start=True, stop=False)
                    nc.tensor.matmul(ps[:, 243:405],
                                     kx_o[:, boff + 128:boff + 256],
                                     qx_o[:, boff + 81:boff + 243],
                                     start=False, stop=True)

                    # decay mask (DVE, psum 1x) -> A^T bf16
                    slot = (4 * q + b) % J
                    at = vat.tile([128, 416], BF16, name="at", tag=f"at{b % 2}")
                    nc.vector.tensor_tensor(
                        at[:, 0:405], ps[:, 0:405],
                        dt_sb[:, slot * 405:slot * 405 + 405], MULT)

                    # AV: out^T [128, 486] = [ht0 l 0:243 | ht1 l 0:243]
                    po = pattn.tile([128, 512], F32, name="ops", tag="attn")
                    v = vsb[hh]
                    for ht in range(2):
                        lhs0 = v[:, bl * 512 + ht * 128:bl * 512 + ht * 128 + 128]
                        lhs1 = v[:, bl * 512 + 256 + ht * 128:
                                 bl * 512 + 256 + ht * 128 + 128]
                        nc.tensor.matmul(po[:, ht * 243:ht * 243 + 243],
                                         lhs0, at[:, 0:243],
                                         start=True, stop=False)
                        nc.tensor.matmul(po[:, ht * 243 + 81:ht * 243 + 243],
                                         lhs1, at[:, 243:405],
                                         start=False, stop=True)

                    # out drain on ACT (DVE is the bottleneck engine)
                    dst = osb[hh][:, bl * 486:bl * 486 + 486]
                    nc.scalar.copy(dst, po[:, 0:486])

                    if bl == 1:
                        nc.sync.dma_start(O_d[2 * q + hh], osb[hh][:])

    nc.compile()
    return nc


def _get_nc():
    if "nc" not in _cache:
        _cache["nc"] = _build()
    return _cache["nc"]


def _run(in_maps, trace=False):
    from concourse import bass_utils
    nc = _get_nc()
    return bass_utils.run_bass_kernel_spmd(
        nc, in_maps, core_ids=list(range(NCORES)), trace=trace)


def kernel(X, W_Q, W_K, W_V, gamma, _trace=False):
    X = np.asarray(X, f32)
    W_all, T_all, DTab = _host_tables(
        np.asarray(W_Q, f32), np.asarray(W_K, f32),
        np.asarray(W_V, f32), np.asarray(gamma, f32))

    in_maps = []
    for c in range(NCORES):
        in_maps.append({
            "X": _host_pack_x(X[c * BPC:(c + 1) * BPC]),
            "WALL": W_all, "TTAB": T_all, "DTAB": DTab,
        })
    res = _run(in_maps, trace=_trace)
    out = np.concatenate([_host_unpack_o(r["OUT"]) for r in res.results],
                         axis=0)
    if _trace:
        _cache["last_result"] = res
    return out.astype(f32)


# revision 14
# speedup vs baseline: 1.5783x; 1.0361x over previous
"""JointRetention Trainium2 kernel.

out[b] = ((xpos(X_b Wq) xpos_down(X_b Wk)^T) * D[b%17]) @ (X_b Wv)

Strategy (v7):
  - Data-parallel over B*J=1088 across 8 cores (136 each; 136%17==0 so the
    joint index pattern is identical on every core).
  - bf16 everywhere (tolerance 2e-2; measured ~1e-3): halves DMA, enables
    FWL weight loads, and 2x DVE perf mode on all SBUF elementwise ops.
  - X host-packed TRANSPOSED (h on partitions) so the PE does zero
    transposes; host also packs xpos cos/sin tables, fused decay tables.
  - Even/odd d-permutation: xpos becomes elementwise muls + half combines.
  - Chunk-sparse scores/AV: D[i,j]=0 for j >= (i//81+1)*81, so score
    m-tile1 skips i<81 and AV accumulates only live (l, m) chunks.
  - Engine balance per quad (4 batches): PE ~9us (proj/V/S/AV matmuls),
    ACT (Q/K/V psum->sbuf bf16 casts + half the out drains), DVE (xpos
    muls at 2x, decay mask, 1 combine, half the out drains), GPSIMD
    (3 of 4 combines).
"""

import numpy as np
from ml_dtypes import bfloat16

L = 243
LP = 244                     # l padded to even for DVE 2x inner dim
H = 256
J = 17
NCORES = 8
NB = 1088
BPC = NB // NCORES           # 136 batches per core
NPAIR = BPC // 2             # 68 pairs per core
NQUAD = NPAIR // 2           # 34 quads per core
SCALE_BASE = 512
CHUNK = 81

f32 = np.float32

_cache = {}


def _host_tables(W_Q, W_K, W_V, gamma):
    half = H // 2
    pe = np.arange(0, H, 2)
    po = np.arange(1, H, 2)
    Wcat = np.concatenate(
        [W_Q[:, pe], W_Q[:, po], W_K[:, pe], W_K[:, po], W_V], axis=1).astype(f32)
    W_all = np.stack([Wcat[0:128], Wcat[128:256]], axis=0)  # (2,128,768)

    base_scale = ((np.arange(0, H, 2, dtype=f32) + 0.4 * H) / (1.4 * H)).astype(f32)
    pos = np.arange(L, dtype=f32)
    scale = base_scale[None, :] ** (pos / SCALE_BASE)[:, None]
    inv_freq = (1.0 / 10000.0 ** (np.arange(half, dtype=f32) / half)).astype(f32)
    sinus = pos[:, None] * inv_freq[None, :]
    sin, cos = np.sin(sinus).astype(f32), np.cos(sinus).astype(f32)
    hCq = (cos * scale).T
    hSq = (sin * scale).T
    hCk = (cos / scale).T
    hSk = (sin / scale).T

    def padl(t):
        out = np.zeros((128, LP), f32)
        out[:, :L] = t
        return out

    def quad_tab(c, s):
        cp = np.concatenate([padl(c), padl(c)], axis=1)
        sp = np.concatenate([padl(s), padl(s)], axis=1)
        return np.concatenate([cp, sp, cp, sp], axis=1)   # (128, 1952)

    T_all = np.stack([quad_tab(hCq, hSq), quad_tab(hSq, hCq),
                      quad_tab(hCk, hSk), quad_tab(hSk, hCk)], axis=0)

    g = gamma.astype(f32)
    i = np.arange(L)[:, None]
    jj = np.arange(L)[None, :]
    allowed = jj < (i // CHUNK + 1) * CHUNK
    absd = np.abs(i - jj).astype(f32)
    D = g[:, None, None] ** absd[None]
    D = np.where(allowed[None], D, 0.0)
    D = np.where(np.isnan(D), 0.0, D).astype(f32)
    DTab = np.zeros((18, 128, 405), f32)
    for s in range(18):
        jt = s % J
        DTab[s, :, 0:L] = D[jt].T[0:128, :]
        DTab[s, 0:L - 128, L:405] = D[jt].T[128:L, 81:L]
    return (W_all.astype(bfloat16), T_all.astype(bfloat16),
            DTab.astype(bfloat16))


def _host_pack_x(Xc):
    # (BPC, 243, 256) f32 -> (NPAIR, 128, 1024) bf16, cols = hc*512+b*256+l
    Xp = Xc.reshape(NPAIR, 2, L, 2, 128)                  # pair, b, l, hc, p
    Xp = np.transpose(Xp, (0, 4, 3, 1, 2))                # pair, p, hc, b, l
    out = np.zeros((NPAIR, 128, 2, 2, 256), f32)
    out[:, :, :, :, 0:L] = Xp
    return np.ascontiguousarray(out.reshape(NPAIR, 128, 1024)).astype(bfloat16)


def _host_unpack_o(Oc):
    # (NPAIR, 128, 972) cols = b*486 + ht*243 + l -> (BPC, 243, 256) f32
    Op = Oc.astype(f32).reshape(NPAIR, 128, 2, 2, L)      # pair, p, b, ht, l
    Op = np.transpose(Op, (0, 2, 4, 3, 1))                # pair, b, l, ht, p
    return np.ascontiguousarray(Op.reshape(BPC, L, H))


def _build():
    import concourse.bacc as bacc
    import concourse.mybir as mybir
    from concourse import tile

    dt = mybir.dt
    F32 = dt.float32
    BF16 = dt.bfloat16
    MULT = mybir.AluOpType.mult
    ADD = mybir.AluOpType.add
    SUB = mybir.AluOpType.subtract

    nc = bacc.Bacc("TRN2", target_bir_lowering=False, debug=False,
                   num_devices=NCORES)
    X_d = nc.dram_tensor("X", (NPAIR, 128, 1024), BF16, kind="ExternalInput").ap()
    W_d = nc.dram_tensor("WALL", (2, 128, 768), BF16, kind="ExternalInput").ap()
    T_d = nc.dram_tensor("TTAB", (4, 128, 1952), BF16, kind="ExternalInput").ap()
    DT_d = nc.dram_tensor("DTAB", (18, 128, 405), BF16, kind="ExternalInput").ap()
    O_d = nc.dram_tensor("OUT", (NPAIR, 128, 972), BF16, kind="ExternalOutput").ap()

    with tile.TileContext(nc) as tc:
        with (
            tc.tile_pool(name="const", bufs=1) as const,
            tc.tile_pool(name="xin", bufs=6) as xin,
            tc.tile_pool(name="eo", bufs=3) as eo_pool,
            tc.tile_pool(name="tab", bufs=3) as tab_pool,
            tc.tile_pool(name="qx", bufs=3) as qx_pool,
            tc.tile_pool(name="vat", bufs=4) as vat,
            tc.tile_pool(name="osb", bufs=4) as osb_pool,
            tc.tile_pool(name="pqk", bufs=2, space="PSUM") as pqk,
            tc.tile_pool(name="pattn", bufs=2, space="PSUM") as pattn,
            tc.tile_pool(name="pout", bufs=1, space="PSUM") as pout,
        ):
            # ---- constants ----
            w_sb = [const.tile([128, 768], BF16, name=f"w{h}", tag=f"w{h}")
                    for h in range(2)]
            t_sb = [const.tile([128, 1952], BF16, name=f"t{i}", tag=f"t{i}")
                    for i in range(4)]
            dt_sb = const.tile([128, 18 * 405], BF16, name="dtab", tag="dtab")
            for h in range(2):
                nc.sync.dma_start(w_sb[h][:], W_d[h])
            for i in range(4):
                nc.sync.dma_start(t_sb[i][:], T_d[i])
            for s in range(18):
                nc.sync.dma_start(dt_sb[:, s * 405:(s + 1) * 405], DT_d[s])

            def load_x(t):
                xi = xin.tile([128, 1024], BF16, name="xi", tag="xi")
                nc.sync.dma_start(xi[:], X_d[t])
                return xi

            xt_cur = load_x(0)

            for q in range(NQUAD):
                qeo = eo_pool.tile([128, 1952], BF16, name="qeo", tag="qeo")
                keo = eo_pool.tile([128, 1952], BF16, name="keo", tag="keo")
                vsb = [None, None]
                for hh in range(2):
                    t = 2 * q + hh
                    xt = xt_cur

                    # ---- proj Q,K: psum [128,1024], e block 0:488, o 512:1000
                    pq_t = pqk.tile([128, 1024], F32, name="pq", tag="pqk")
                    pk_t = pqk.tile([128, 1024], F32, name="pk", tag="pqk")
                    for ti, ps in ((0, pq_t), (1, pk_t)):
                        for eo in range(2):
                            for hc in range(2):
                                mov = xt[:, hc * 512:hc * 512 + 512].rearrange(
                                    "p (b l) -> p b l", b=2)[:, :, 0:LP]
                                nc.tensor.matmul(
                                    ps[:, eo * 512:eo * 512 + 488],
                                    w_sb[hc][:, (ti * 2 + eo) * 128:
                                             (ti * 2 + eo) * 128 + 128],
                                    mov,
                                    start=(hc == 0), stop=(hc == 1),
                                )

                    # ---- V: psum [128,1024] = [b0 m0 | b0 m1 | b1 m0 | b1 m1]
                    pv_t = pqk.tile([128, 1024], F32, name="pv", tag="pqk")
                    for b in range(2):
                        for mc in range(2):
                            for hc in range(2):
                                off = hc * 512 + b * 256 + mc * 128
                                nc.tensor.matmul(
                                    pv_t[:, b * 512 + mc * 256:
                                         b * 512 + mc * 256 + 256],
                                    xt[:, off:off + 128],
                                    w_sb[hc][:, 512:768],
                                    start=(hc == 0), stop=(hc == 1),
                                )

                    # prefetch next pair's X
                    if t + 1 < NPAIR:
                        xt_cur = load_x(t + 1)

                    # ---- ACT drains (psum f32 -> sbuf bf16)
                    src = pq_t[:].rearrange("p (e c) -> p e c", e=2)[:, :, 0:488]
                    dst = qeo[:, hh * 976:hh * 976 + 976].rearrange(
                        "p (e c) -> p e c", e=2)
                    nc.scalar.copy(dst, src)
                    src = pk_t[:].rearrange("p (e c) -> p e c", e=2)[:, :, 0:488]
                    dst = keo[:, hh * 976:hh * 976 + 976].rearrange(
                        "p (e c) -> p e c", e=2)
                    nc.scalar.copy(dst, src)
                    vt = vat.tile([128, 1024], BF16, name="vsb", tag=f"vsb{hh}")
                    nc.scalar.copy(vt[:], pv_t[:])
                    vsb[hh] = vt

                # ---- xpos muls (DVE, bf16 2x) ----
                ta_q = tab_pool.tile([128, 1952], BF16, name="taq", tag="taq")
                tb_q = tab_pool.tile([128, 1952], BF16, name="tbq", tag="tbq")
                ta_k = tab_pool.tile([128, 1952], BF16, name="tak", tag="tak")
                tb_k = tab_pool.tile([128, 1952], BF16, name="tbk", tag="tbk")
                nc.vector.tensor_tensor(ta_q[:], qeo[:], t_sb[0][:], MULT)
                nc.vector.tensor_tensor(tb_q[:], qeo[:], t_sb[1][:], MULT)
                nc.vector.tensor_tensor(ta_k[:], keo[:], t_sb[2][:], MULT)
                nc.vector.tensor_tensor(tb_k[:], keo[:], t_sb[3][:], MULT)

                # ---- combines -> qx/kx [128,1024] 256-strided (pads stay 0)
                qx_e = qx_pool.tile([128, 1024], BF16, name="qxe", tag="qxe")
                qx_o = qx_pool.tile([128, 1024], BF16, name="qxo", tag="qxo")
                kx_e = qx_pool.tile([128, 1024], BF16, name="kxe", tag="kxe")
                kx_o = qx_pool.tile([128, 1024], BF16, name="kxo", tag="kxo")
                # l-pad columns of kx feed S stationary slices; keep them zero
                # (qx is only ever read as moving operand over real columns)
                for z in (kx_e, kx_o):
                    nc.gpsimd.memset(
                        z[:].rearrange("p (g l) -> p g l", g=4)[:, :, LP:256], 0.0)

                def c_src(tab, eo):
                    a = tab[:].rearrange("p (pr c) -> p pr c", pr=2)
                    a = a[:, :, eo * 488:(eo + 1) * 488]
                    return a.rearrange("p pr (b l) -> p pr b l", b=2)

                def c_dst(dst):
                    return dst[:].rearrange(
                        "p (pr b l) -> p pr b l", pr=2, b=2)[:, :, :, 0:LP]

                # all combines on DVE: gpsimd elementwise halves DVE
                # throughput via the shared SBUF port (measured)
                nc.vector.tensor_tensor(
                    c_dst(qx_e), c_src(ta_q, 0), c_src(ta_q, 1), SUB)
                nc.vector.tensor_tensor(
                    c_dst(qx_o), c_src(tb_q, 1), c_src(tb_q, 0), ADD)
                nc.vector.tensor_tensor(
                    c_dst(kx_e), c_src(ta_k, 0), c_src(ta_k, 1), SUB)
                nc.vector.tensor_tensor(
                    c_dst(kx_o), c_src(tb_k, 1), c_src(tb_k, 0), ADD)

                # ---- attention per batch ----
                osb = [None, None]
                po = None
                for b in range(4):
                    hh = b // 2
                    bl = b % 2
                    if bl == 0:
                        osb[hh] = osb_pool.tile([128, 972], BF16,
                                                name="ob", tag=f"ob{hh}")
                        po = pout.tile([128, 1024], F32, name="ops", tag="outp")
                    boff = b * 256
                    # scores S^T [128, 405]: mt0 cols 0:243, mt1 243:405
                    ps = pattn.tile([128, 512], F32, name="sps", tag="attn")
                    nc.tensor.matmul(ps[:, 0:243],
                                     kx_e[:, boff:boff + 128],
                                     qx_e[:, boff:boff + 243],
                                     start=True, stop=False)
                    nc.tensor.matmul(ps[:, 0:243],
                                     kx_o[:, boff:boff + 128],
                                     qx_o[:, boff:boff + 243],
                                     start=False, stop=True)
                    nc.tensor.matmul(ps[:, 243:405],
                                     kx_e[:, boff + 128:boff + 256],
                                     qx_e[:, boff + 81:boff + 243],
                                     start=True, stop=False)
                    nc.tensor.matmul(ps[:, 243:405],
                                     kx_o[:, boff + 128:boff + 256],
                                     qx_o[:, boff + 81:boff + 243],
                                     start=False, stop=True)

                    # decay mask (DVE, psum 1x) -> A^T bf16
                    slot = (4 * q + b) % J
                    at = vat.tile([128, 416], BF16, name="at", tag=f"at{b % 2}")
                    nc.vector.tensor_tensor(
                        at[:, 0:405], ps[:, 0:405],
                        dt_sb[:, slot * 405:slot * 405 + 405], MULT)

                    # AV: out^T [128, 486] = [ht0 l 0:243 | ht1 l 0:243]
                    v = vsb[hh]
                    for ht in range(2):
                        lhs0 = v[:, bl * 512 + ht * 128:bl * 512 + ht * 128 + 128]
                        lhs1 = v[:, bl * 512 + 256 + ht * 128:
                                 bl * 512 + 256 + ht * 128 + 128]
                        base = bl * 512 + ht * 243
                        nc.tensor.matmul(po[:, base:base + 243],
                                         lhs0, at[:, 0:243],
                                         start=True, stop=False)
                        nc.tensor.matmul(po[:, base + 81:base + 243],
                                         lhs1, at[:, 243:405],
                                         start=False, stop=True)

                    if bl == 1:
                        # one merged out drain per pair on ACT
                        src = po[:].rearrange("p (b c) -> p b c", b=2)[:, :, 0:486]
                        dst = osb[hh][:].rearrange("p (b c) -> p b c", b=2)
                        nc.scalar.copy(dst, src)
                        nc.sync.dma_start(O_d[2 * q + hh], osb[hh][:])

    nc.compile()
    return nc


def _get_nc():
    if "nc" not in _cache:
        _cache["nc"] = _build()
    return _cache["nc"]


def _run(in_maps, trace=False):
    from concourse import bass_utils
    nc = _get_nc()
    return bass_utils.run_bass_kernel_spmd(
        nc, in_maps, core_ids=list(range(NCORES)), trace=trace)


def kernel(X, W_Q, W_K, W_V, gamma, _trace=False):
    X = np.asarray(X, f32)
    W_all, T_all, DTab = _host_tables(
        np.asarray(W_Q, f32), np.asarray(W_K, f32),
        np.asarray(W_V, f32), np.asarray(gamma, f32))

    in_maps = []
    for c in range(NCORES):
        in_maps.append({
            "X": _host_pack_x(X[c * BPC:(c + 1) * BPC]),
            "WALL": W_all, "TTAB": T_all, "DTAB": DTab,
        })
    res = _run(in_maps, trace=_trace)
    out = np.concatenate([_host_unpack_o(r["OUT"]) for r in res.results],
                         axis=0)
    if _trace:
        _cache["last_result"] = res
    return out.astype(f32)
